# revision 13
# baseline (speedup 1.0000x reference)
"""EyesMouthLoss Trainium2 kernel (v3).

loss = mean(|pred-target| * (1 + 299*clip(eye_mask+mouth_mask, 0, 1)))

Sharding: pure data-parallel over B=16 -> 2 batches per core on 8 cores.
Host sums the 8 per-core partial scalars (the final all-reduce).

Design (v3):
- Math refactor: loss*N = sum(|d * V|) with V = 1 + min(299*(e'+m'), 299)
  and e', m' the 299-prescaled radial max-fields.  One fp32 accumulator
  stream (ACT Abs accum_out).
- Host-precomputed per-core stencil bank (fp64 sqrt, only that core's
  ~60 deduped row-shifts) DMA'd in first; no on-device bank build, no
  ACT table thrash (Abs only).
- Loads are HWDGE fp32 DMAs split across both hardware DGE rings
  (pred on the sync/SP ring, targ on the scalar/Activation ring): RTL
  descriptor generation, so the stream saturates HBM from ~1us instead
  of being paced by the Q7's ~1.1us-per-DMA software emission.
- Per unit: sub (DVE, fp32->bf16 1x), mult by V broadcast (DVE, bf16
  2x), Abs+accum (ACT).  V finalize (add + min/+1) runs on the
  otherwise-idle GpSimd.  Unit-major tail enforced with dep hints.
"""

import sys

sys.path.insert(0, "/opt/trn_rl_repo")

from contextlib import ExitStack

import numpy as np

import concourse.bass as bass
import concourse.tile as tile
from concourse import bacc, mybir
from concourse.bass_utils import run_bass_kernel_spmd
from concourse.tile import add_dep_helper

B, C, H, W = 16, 3, 512, 512
NCORES = 8
BPC = B // NCORES  # batches per core
RADIUS = 15.0
HALF = 14  # region strictly zero for |dx| >= 15
WIN = 2 * HALF + 1  # 29
EYE = (36, 48)
MOUTH = (48, 68)
WEIGHT = 300.0
NTOT = float(B * C * H * W)
FP32 = mybir.dt.float32
BF16 = mybir.dt.bfloat16
Alu = mybir.AluOpType
Act = mybir.ActivationFunctionType

# unit = (bi, k, x0, xn); last chunk split in half for a short tail
UNITS = (
    [(0, k, 0, W) for k in range(4)]
    + [(1, k, 0, W) for k in range(3)]
    + [(1, 3, 0, W // 2), (1, 3, W // 2, W // 2)]
)
NU = len(UNITS)


def _windows_for(lm_b, lo, hi):
    """Window pieces (t, k, x0, ncols, sc0) for one landmark group."""
    pieces = []
    seen = set()
    for cx, cy in lm_b[lo:hi]:
        cx = int(min(max(int(cx), 0), W - 1))
        cy = int(min(max(int(cy), 0), H - 1))
        if (cx, cy) in seen:
            continue
        seen.add((cx, cy))
        y0, y1 = max(0, cy - HALF), min(H - 1, cy + HALF)
        x0, x1 = max(0, cx - HALF), min(W - 1, cx + HALF)
        sc0 = x0 - (cx - HALF)
        ncols = x1 - x0 + 1
        for k in range(y0 >> 7, (y1 >> 7) + 1):
            t = cy - 128 * k + 14
            pieces.append((t, k, x0, ncols, sc0))
    return pieces


def _host_windows_and_banks(landmarks):
    """Per-core window piece lists (with dense shift indices) + stencil banks.

    bank[core][p, ti, j] = 299 * (1 - sqrt((p - t + 14)^2 + (j - 14)^2)/15)
    for t = shifts[core][ti]; negative outside the radius = max-neutral
    against the zero-initialized fields.
    """
    core_pieces = []  # [core][bi][field] -> list[(ti, k, x0, ncols, sc0)]
    core_shifts = []
    for core in range(NCORES):
        shifts = set()
        raw = []
        for bi in range(BPC):
            lm_b = landmarks[core * BPC + bi]
            per_field = []
            for lo, hi in (EYE, MOUTH):
                ps = _windows_for(lm_b, lo, hi)
                shifts.update(t for t, *_ in ps)
                per_field.append(ps)
            raw.append(per_field)
        slist = sorted(shifts)
        tmap = {t: i for i, t in enumerate(slist)}
        core_pieces.append(
            [
                [[(tmap[t], k, x0, nc_, sc0) for t, k, x0, nc_, sc0 in ps] for ps in pf]
                for pf in raw
            ]
        )
        core_shifts.append(slist)

    tmax = max(len(s) for s in core_shifts)
    banks = np.zeros((NCORES, 128, tmax, WIN), dtype=np.float32)
    p = np.arange(128.0)[:, None]
    j = np.arange(float(WIN))[None, :]
    for core, slist in enumerate(core_shifts):
        for ti, t in enumerate(slist):
            dist = np.sqrt((p - t + 14.0) ** 2 + (j - 14.0) ** 2)
            banks[core, :, ti, :] = (WEIGHT - 1.0) * (1.0 - dist / RADIUS)
    return core_pieces, banks.astype(mybir.dt.np(BF16)), tmax


def _build(core_pieces, tmax):
    """Build the SPMD Bass program, specialized to the landmark values."""
    nc = bacc.Bacc(None)
    pred_p = nc.declare_dram_parameter("pred", [BPC, C, H, W], FP32, isOutput=False)
    targ_p = nc.declare_dram_parameter("targ", [BPC, C, H, W], FP32, isOutput=False)
    bank_p = nc.declare_dram_parameter("bank", [128, tmax, WIN], BF16, isOutput=False)
    out_p = nc.declare_dram_parameter("out", [128, NU], FP32, isOutput=True)

    with tile.TileContext(nc) as tc, ExitStack() as ctx:
        stat_pool = ctx.enter_context(tc.tile_pool(name="stat", bufs=1))
        const_pool = ctx.enter_context(tc.tile_pool(name="const", bufs=1))
        load_pool = ctx.enter_context(tc.tile_pool(name="load", bufs=2))
        field_pool = ctx.enter_context(tc.tile_pool(name="field", bufs=2))

        # ---- tiles; field zeroing split across ACT (batch 0, ~1us each,
        # before the targ DMA emissions) and the Q7 (batch 1, in parallel) so
        # both batches' fields are ready by ~3us and the windows -> V chain
        # clears the DVE queue early ----
        tiles = []
        for bi in range(BPC):
            p_t = load_pool.tile([128, C, 4, W], FP32, tag="p_t", name=f"p_t{bi}")
            t_t = load_pool.tile([128, C, 4, W], FP32, tag="t_t", name=f"t_t{bi}")
            d_t = load_pool.tile([128, C, 4, W], BF16, tag="d_t", name=f"d_t{bi}")
            e_f = field_pool.tile([128, 4, W], BF16, tag="e_f", name=f"e_f{bi}")
            m_f = field_pool.tile([128, 4, W], BF16, tag="m_f", name=f"m_f{bi}")
            if bi == 0:
                nc.scalar.memzero(e_f[:])
                nc.scalar.memzero(m_f[:])
            else:
                nc.gpsimd.memset(e_f[:], 0.0)
                nc.gpsimd.memset(m_f[:], 0.0)
            tiles.append((p_t, t_t, d_t, e_f, m_f))

        # ---- host-precomputed stencil bank: first on the sync ring so the
        # windows unblock immediately ----
        bank_t = const_pool.tile([128, tmax, WIN], BF16)
        nc.sync.dma_start(bank_t[:], bank_p[:])

        # ---- HWDGE fp32 loads: pred on the sync(SP) ring, targ on the
        # scalar(Activation) ring -- two parallel RTL descriptor generators,
        # so the stream saturates HBM from ~1.5us ----
        for bi, k, x0, xn in UNITS:
            p_t, t_t, _, _, _ = tiles[bi]
            rows = slice(128 * k, 128 * (k + 1))
            cols = slice(x0, x0 + xn)
            nc.sync.dma_start(
                p_t[:, :, k, cols],
                pred_p[bi, :, rows, cols].rearrange("c p x -> p c x"),
            )
            nc.scalar.dma_start(
                t_t[:, :, k, cols],
                targ_p[bi, :, rows, cols].rearrange("c p x -> p c x"),
            )

        # partition id + dispatch-prefetch hint AFTER the load issues
        core_idx = nc.vector.partition_id()
        win_hint = nc.vector.switch_hint(core_idx, NCORES, label="win")

        # pre-switch DVE touches of every cross-engine input the switch body
        # reads (fields from Q7 memsets, bank from the sync DMA): the waits
        # land here, outside the branch, and are elided inside it -- a
        # cross-engine wait inside a Switch case deadlocks the scheduler
        touch = stat_pool.tile([128, 1], BF16)
        for bi in range(BPC):
            _, _, _, e_f, m_f = tiles[bi]
            nc.vector.tensor_tensor(
                touch[:], e_f[:, 0, 0:1], m_f[:, 0, 0:1], op=Alu.max
            )
        nc.vector.tensor_tensor(touch[:], touch[:], bank_t[:, 0, 0:1], op=Alu.max)

        # ---- per-core landmark windows, one Switch for both batches ----
        for case in tc.Switch(core_idx, NCORES, hint=win_hint):
            for bi in range(BPC):
                _, _, _, e_f, m_f = tiles[bi]
                for field, pieces in zip((e_f, m_f), core_pieces[case][bi]):
                    for ti, k, x0, ncols, sc0 in pieces:
                        nc.vector.tensor_tensor(
                            field[:, k, x0 : x0 + ncols],
                            field[:, k, x0 : x0 + ncols],
                            bank_t[:, ti, sc0 : sc0 + ncols],
                            op=Alu.max,
                        )

        # ---- V = 1 + min(e'+m', 299), in place into e_f (DVE: the add runs
        # 2x, the fused min/+1 tensor_scalar runs 4x) ----
        for bi in range(BPC):
            _, _, _, e_f, m_f = tiles[bi]
            nc.vector.tensor_tensor(e_f[:], e_f[:], m_f[:], op=Alu.add)
            nc.vector.tensor_scalar(
                e_f[:], e_f[:], WEIGHT - 1.0, 1.0, op0=Alu.min, op1=Alu.add
            )

        # ---- per-unit pipeline: d = p-t (fp32 in, bf16 out); d *= V (bf16
        # 2x); |d| on ACT with fp32 row-sums ----
        rs = stat_pool.tile([128, NU], FP32)
        prev_mult = None
        for u, (bi, k, x0, xn) in enumerate(UNITS):
            p_t, t_t, d_t, e_f, _ = tiles[bi]
            cols = slice(x0, x0 + xn)
            dp = d_t[:, :, k, cols]
            sub = nc.vector.tensor_tensor(
                dp, p_t[:, :, k, cols], t_t[:, :, k, cols], op=Alu.subtract
            )
            if prev_mult is not None:
                # pin strict unit-major order on the DVE: each unit's MULTIPLY
                # runs right after its SUBTRACT, so mults stream with the data
                # instead of piling up behind a late V / data-blocked sub
                add_dep_helper(sub.ins, prev_mult.ins, reason="unit-major order")
            v3 = (
                e_f[:, k, cols]
                .broadcast_to([128, xn, C])
                .rearrange("p x c -> p c x")
            )
            prev_mult = nc.vector.tensor_tensor(dp, dp, v3, op=Alu.mult)
            nc.scalar.activation(dp, dp, Act.Abs, accum_out=rs[:, u : u + 1])
            if u == NU - 2:
                # all but the last tail column: overlap the big out-DMA
                nc.sync.dma_start(out_p[:, 0 : NU - 1], rs[:, 0 : NU - 1])
        nc.sync.dma_start(out_p[:, NU - 1 : NU], rs[:, NU - 1 : NU])

    return nc


def run(inputs, trace=False):
    pred = np.ascontiguousarray(inputs["pred"], dtype=np.float32)
    targ = np.ascontiguousarray(inputs["target"], dtype=np.float32)
    lms = np.asarray(inputs["landmarks"])
    assert pred.shape == (B, C, H, W) and targ.shape == (B, C, H, W)

    core_pieces, banks_bf16, tmax = _host_windows_and_banks(lms)
    nc = _build(core_pieces, tmax)
    nc.finalize()
    in_maps = [
        {
            "pred": pred[i * BPC : (i + 1) * BPC],
            "targ": targ[i * BPC : (i + 1) * BPC],
            "bank": banks_bf16[i],
        }
        for i in range(NCORES)
    ]
    res = run_bass_kernel_spmd(nc, in_maps, list(range(NCORES)), trace=trace)
    total = 0.0
    for i in range(NCORES):
        total += res.results[i]["out"].astype(np.float64).sum()
    return np.float32(total / NTOT), res


def kernel(pred, target, landmarks):
    out, _ = run({"pred": pred, "target": target, "landmarks": landmarks})
    return out


# revision 16
# speedup vs baseline: 1.0333x; 1.0333x over previous
"""EyesMouthLoss Trainium2 kernel (v3).

loss = mean(|pred-target| * (1 + 299*clip(eye_mask+mouth_mask, 0, 1)))

Sharding: pure data-parallel over B=16 -> 2 batches per core on 8 cores.
Host sums the 8 per-core partial scalars (the final all-reduce).

Design (v3):
- Math refactor: loss*N = sum(|d * V|) with V = 1 + min(299*(e'+m'), 299)
  and e', m' the 299-prescaled radial max-fields.  One fp32 accumulator
  stream (ACT Abs accum_out).
- Host-precomputed per-core stencil bank (fp64 sqrt, only that core's
  ~60 deduped row-shifts) DMA'd in first; no on-device bank build, no
  ACT table thrash (Abs only).
- Loads are HWDGE fp32 DMAs split across both hardware DGE rings
  (pred on the sync/SP ring, targ on the scalar/Activation ring): RTL
  descriptor generation, so the stream saturates HBM from ~1us instead
  of being paced by the Q7's ~1.1us-per-DMA software emission.
- Per unit: sub (DVE, fp32->bf16 1x), mult by V broadcast (DVE, bf16
  2x), Abs+accum (ACT).  V finalize (add + min/+1) runs on the
  otherwise-idle GpSimd.  Unit-major tail enforced with dep hints.
"""

import sys

sys.path.insert(0, "/opt/trn_rl_repo")

from contextlib import ExitStack

import numpy as np

import concourse.bass as bass
import concourse.tile as tile
from concourse import bacc, mybir
from concourse.bass_utils import run_bass_kernel_spmd
from concourse.tile import add_dep_helper

B, C, H, W = 16, 3, 512, 512
NCORES = 8
BPC = B // NCORES  # batches per core
RADIUS = 15.0
HALF = 14  # region strictly zero for |dx| >= 15
WIN = 2 * HALF + 1  # 29
EYE = (36, 48)
MOUTH = (48, 68)
WEIGHT = 300.0
NTOT = float(B * C * H * W)
FP32 = mybir.dt.float32
BF16 = mybir.dt.bfloat16
Alu = mybir.AluOpType
Act = mybir.ActivationFunctionType

# unit = (bi, k, x0, xn); last chunk split in half for a short tail
UNITS = (
    [(0, k, 0, W) for k in range(4)]
    + [(1, k, 0, W) for k in range(3)]
    + [(1, 3, 0, W // 2), (1, 3, W // 2, W // 2)]
)
NU = len(UNITS)


def _windows_for(lm_b, lo, hi):
    """Window pieces (t, k, x0, ncols, sc0) for one landmark group."""
    pieces = []
    seen = set()
    for cx, cy in lm_b[lo:hi]:
        cx = int(min(max(int(cx), 0), W - 1))
        cy = int(min(max(int(cy), 0), H - 1))
        if (cx, cy) in seen:
            continue
        seen.add((cx, cy))
        y0, y1 = max(0, cy - HALF), min(H - 1, cy + HALF)
        x0, x1 = max(0, cx - HALF), min(W - 1, cx + HALF)
        sc0 = x0 - (cx - HALF)
        ncols = x1 - x0 + 1
        for k in range(y0 >> 7, (y1 >> 7) + 1):
            t = cy - 128 * k + 14
            pieces.append((t, k, x0, ncols, sc0))
    return pieces


def _host_windows_and_banks(landmarks):
    """Per-core window piece lists (with dense shift indices) + stencil banks.

    bank[core][p, ti, j] = 299 * (1 - sqrt((p - t + 14)^2 + (j - 14)^2)/15)
    for t = shifts[core][ti]; negative outside the radius = max-neutral
    against the zero-initialized fields.
    """
    core_pieces = []  # [core][bi][field] -> list[(ti, k, x0, ncols, sc0)]
    core_shifts = []
    for core in range(NCORES):
        shifts = set()
        raw = []
        for bi in range(BPC):
            lm_b = landmarks[core * BPC + bi]
            per_field = []
            for lo, hi in (EYE, MOUTH):
                ps = _windows_for(lm_b, lo, hi)
                shifts.update(t for t, *_ in ps)
                per_field.append(ps)
            raw.append(per_field)
        slist = sorted(shifts)
        tmap = {t: i for i, t in enumerate(slist)}
        core_pieces.append(
            [
                [[(tmap[t], k, x0, nc_, sc0) for t, k, x0, nc_, sc0 in ps] for ps in pf]
                for pf in raw
            ]
        )
        core_shifts.append(slist)

    tmax = max(len(s) for s in core_shifts)
    banks = np.zeros((NCORES, 128, tmax, WIN), dtype=np.float32)
    p = np.arange(128.0)[:, None]
    j = np.arange(float(WIN))[None, :]
    for core, slist in enumerate(core_shifts):
        for ti, t in enumerate(slist):
            dist = np.sqrt((p - t + 14.0) ** 2 + (j - 14.0) ** 2)
            banks[core, :, ti, :] = (WEIGHT - 1.0) * (1.0 - dist / RADIUS)
    return core_pieces, banks.astype(mybir.dt.np(BF16)), tmax


def _build(core_pieces, tmax):
    """Build the SPMD Bass program, specialized to the landmark values."""
    nc = bacc.Bacc(None)
    pred_p = nc.declare_dram_parameter("pred", [BPC, C, H, W], FP32, isOutput=False)
    targ_p = nc.declare_dram_parameter("targ", [BPC, C, H, W], FP32, isOutput=False)
    bank_p = nc.declare_dram_parameter("bank", [128, tmax, WIN], BF16, isOutput=False)
    out_p = nc.declare_dram_parameter("out", [128, NU], FP32, isOutput=True)

    with tile.TileContext(nc) as tc, ExitStack() as ctx:
        stat_pool = ctx.enter_context(tc.tile_pool(name="stat", bufs=1))
        const_pool = ctx.enter_context(tc.tile_pool(name="const", bufs=1))
        load_pool = ctx.enter_context(tc.tile_pool(name="load", bufs=2))
        field_pool = ctx.enter_context(tc.tile_pool(name="field", bufs=2))

        # ---- tiles; field zeroing split across ACT (batch 0, ~1us each,
        # before the targ DMA emissions) and the Q7 (batch 1, in parallel) so
        # both batches' fields are ready by ~3us and the windows -> V chain
        # clears the DVE queue early ----
        tiles = []
        for bi in range(BPC):
            p_t = load_pool.tile([128, C, 4, W], FP32, tag="p_t", name=f"p_t{bi}")
            t_t = load_pool.tile([128, C, 4, W], FP32, tag="t_t", name=f"t_t{bi}")
            d_t = load_pool.tile([128, C, 4, W], BF16, tag="d_t", name=f"d_t{bi}")
            e_f = field_pool.tile([128, 4, W], BF16, tag="e_f", name=f"e_f{bi}")
            m_f = field_pool.tile([128, 4, W], BF16, tag="m_f", name=f"m_f{bi}")
            if bi == 0:
                nc.scalar.memzero(e_f[:])
                nc.scalar.memzero(m_f[:])
            else:
                nc.gpsimd.memset(e_f[:], 0.0)
                nc.gpsimd.memset(m_f[:], 0.0)
            tiles.append((p_t, t_t, d_t, e_f, m_f))

        # ---- host-precomputed stencil bank: first on the sync ring so the
        # windows unblock immediately ----
        bank_t = const_pool.tile([128, tmax, WIN], BF16)
        nc.sync.dma_start(bank_t[:], bank_p[:])

        # ---- HWDGE fp32 loads: pred on the sync(SP) ring, targ on the
        # scalar(Activation) ring -- two parallel RTL descriptor generators,
        # so the stream saturates HBM from ~1.5us ----
        t_dmas = []
        for bi, k, x0, xn in UNITS:
            p_t, t_t, _, _, _ = tiles[bi]
            rows = slice(128 * k, 128 * (k + 1))
            cols = slice(x0, x0 + xn)
            nc.sync.dma_start(
                p_t[:, :, k, cols],
                pred_p[bi, :, rows, cols].rearrange("c p x -> p c x"),
            )
            t_dmas.append(nc.scalar.dma_start(
                t_t[:, :, k, cols],
                targ_p[bi, :, rows, cols].rearrange("c p x -> p c x"),
            ))

        # partition id + dispatch-prefetch hint AFTER the load issues
        core_idx = nc.vector.partition_id()
        win_hint = nc.vector.switch_hint(core_idx, NCORES, label="win")

        # pre-switch DVE touches of every cross-engine input the switch body
        # reads (fields from Q7 memsets, bank from the sync DMA): the waits
        # land here, outside the branch, and are elided inside it -- a
        # cross-engine wait inside a Switch case deadlocks the scheduler
        touch = stat_pool.tile([128, 1], BF16)
        for bi in range(BPC):
            _, _, _, e_f, m_f = tiles[bi]
            nc.vector.tensor_tensor(
                touch[:], e_f[:, 0, 0:1], m_f[:, 0, 0:1], op=Alu.max
            )
        nc.vector.tensor_tensor(touch[:], touch[:], bank_t[:, 0, 0:1], op=Alu.max)

        # ---- per-core landmark windows, one Switch for both batches ----
        for case in tc.Switch(core_idx, NCORES, hint=win_hint):
            for bi in range(BPC):
                _, _, _, e_f, m_f = tiles[bi]
                for field, pieces in zip((e_f, m_f), core_pieces[case][bi]):
                    for ti, k, x0, ncols, sc0 in pieces:
                        nc.vector.tensor_tensor(
                            field[:, k, x0 : x0 + ncols],
                            field[:, k, x0 : x0 + ncols],
                            bank_t[:, ti, sc0 : sc0 + ncols],
                            op=Alu.max,
                        )

        # ---- V = 1 + min(e'+m', 299), in place into e_f (DVE: the add runs
        # 2x, the fused min/+1 tensor_scalar runs 4x) ----
        v_ts = []
        for bi in range(BPC):
            _, _, _, e_f, m_f = tiles[bi]
            nc.vector.tensor_tensor(e_f[:], e_f[:], m_f[:], op=Alu.add)
            v_ts.append(nc.vector.tensor_scalar(
                e_f[:], e_f[:], WEIGHT - 1.0, 1.0, op0=Alu.min, op1=Alu.add
            ))

        # ---- per-unit pipeline: d = p-t (fp32 in, bf16 out); d *= V (bf16
        # 2x); |d| on ACT with fp32 row-sums ----
        rs = stat_pool.tile([128, NU], FP32)
        prev_mult = None
        for u, (bi, k, x0, xn) in enumerate(UNITS):
            p_t, t_t, d_t, e_f, _ = tiles[bi]
            cols = slice(x0, x0 + xn)
            dp = d_t[:, :, k, cols]
            sub = nc.vector.tensor_tensor(
                dp, p_t[:, :, k, cols], t_t[:, :, k, cols], op=Alu.subtract
            )
            if u >= NU - 2 and prev_mult is not None:
                # tail: pin unit-major order so the last ready MULTIPLY isn't
                # queued behind a data-blocked SUBTRACT
                add_dep_helper(sub.ins, prev_mult.ins, reason="unit-major tail")
            if u == 1:
                # force the windows->V chain for batch 0 to clear the DVE
                # queue before the unit stream takes over; otherwise V lands
                # late and every MULTIPLY serializes at the tail
                add_dep_helper(sub.ins, v_ts[0].ins, reason="V0 before units")
            if u == 5:
                add_dep_helper(sub.ins, v_ts[1].ins, reason="V1 before batch1")
            v3 = (
                e_f[:, k, cols]
                .broadcast_to([128, xn, C])
                .rearrange("p x c -> p c x")
            )
            prev_mult = nc.vector.tensor_tensor(dp, dp, v3, op=Alu.mult)
            ab = nc.scalar.activation(dp, dp, Act.Abs, accum_out=rs[:, u : u + 1])
            if u == 0:
                # all targ-DMA emissions on the ACT ring must precede the
                # first Abs, or the scheduler defers them and the late units
                # starve waiting for data whose DMA was never issued
                for td in t_dmas:
                    add_dep_helper(ab.ins, td.ins, reason="t emissions first")
            if u == NU - 2:
                # all but the last tail column: overlap the big out-DMA
                nc.sync.dma_start(out_p[:, 0 : NU - 1], rs[:, 0 : NU - 1])
        nc.sync.dma_start(out_p[:, NU - 1 : NU], rs[:, NU - 1 : NU])

    return nc


def run(inputs, trace=False):
    pred = np.ascontiguousarray(inputs["pred"], dtype=np.float32)
    targ = np.ascontiguousarray(inputs["target"], dtype=np.float32)
    lms = np.asarray(inputs["landmarks"])
    assert pred.shape == (B, C, H, W) and targ.shape == (B, C, H, W)

    core_pieces, banks_bf16, tmax = _host_windows_and_banks(lms)
    nc = _build(core_pieces, tmax)
    nc.finalize()
    in_maps = [
        {
            "pred": pred[i * BPC : (i + 1) * BPC],
            "targ": targ[i * BPC : (i + 1) * BPC],
            "bank": banks_bf16[i],
        }
        for i in range(NCORES)
    ]
    res = run_bass_kernel_spmd(nc, in_maps, list(range(NCORES)), trace=trace)
    total = 0.0
    for i in range(NCORES):
        total += res.results[i]["out"].astype(np.float64).sum()
    return np.float32(total / NTOT), res


def kernel(pred, target, landmarks):
    out, _ = run({"pred": pred, "target": target, "landmarks": landmarks})
    return out


# revision 18
# speedup vs baseline: 1.1920x; 1.1536x over previous
"""EyesMouthLoss Trainium2 kernel (v3).

loss = mean(|pred-target| * (1 + 299*clip(eye_mask+mouth_mask, 0, 1)))

Sharding: pure data-parallel over B=16 -> 2 batches per core on 8 cores.
Host sums the 8 per-core partial scalars (the final all-reduce).

Design (v3):
- Math refactor: loss*N = sum(|d * V|) with V = 1 + min(299*(e'+m'), 299)
  and e', m' the 299-prescaled radial max-fields.  One fp32 accumulator
  stream (ACT Abs accum_out).
- Host-precomputed per-core stencil bank (fp64 sqrt, only that core's
  ~60 deduped row-shifts) DMA'd in first; no on-device bank build, no
  ACT table thrash (Abs only).
- Loads are HWDGE fp32 DMAs split across both hardware DGE rings
  (pred on the sync/SP ring, targ on the scalar/Activation ring): RTL
  descriptor generation, so the stream saturates HBM from ~1us instead
  of being paced by the Q7's ~1.1us-per-DMA software emission.
- Per unit: sub (DVE, fp32->bf16 1x), mult by V broadcast (DVE, bf16
  2x), Abs+accum (ACT).  V finalize (add + min/+1) runs on the
  otherwise-idle GpSimd.  Unit-major tail enforced with dep hints.
"""

import sys

sys.path.insert(0, "/opt/trn_rl_repo")

from contextlib import ExitStack

import numpy as np

import concourse.bass as bass
import concourse.tile as tile
from concourse import bacc, mybir
from concourse.bass_utils import run_bass_kernel_spmd
from concourse.tile import add_dep_helper

B, C, H, W = 16, 3, 512, 512
NCORES = 8
BPC = B // NCORES  # batches per core
RADIUS = 15.0
HALF = 14  # region strictly zero for |dx| >= 15
WIN = 2 * HALF + 1  # 29
EYE = (36, 48)
MOUTH = (48, 68)
WEIGHT = 300.0
NTOT = float(B * C * H * W)
FP32 = mybir.dt.float32
BF16 = mybir.dt.bfloat16
Alu = mybir.AluOpType
Act = mybir.ActivationFunctionType

# unit = (bi, k, x0, xn); last chunk split in half for a short tail
UNITS = (
    [(0, k, 0, W) for k in range(4)]
    + [(1, k, 0, W) for k in range(3)]
    + [(1, 3, 0, W // 2), (1, 3, W // 2, W // 2)]
)
NU = len(UNITS)


def _windows_for(lm_b, lo, hi):
    """Window pieces (t, k, x0, ncols, sc0) for one landmark group."""
    pieces = []
    seen = set()
    for cx, cy in lm_b[lo:hi]:
        cx = int(min(max(int(cx), 0), W - 1))
        cy = int(min(max(int(cy), 0), H - 1))
        if (cx, cy) in seen:
            continue
        seen.add((cx, cy))
        y0, y1 = max(0, cy - HALF), min(H - 1, cy + HALF)
        x0, x1 = max(0, cx - HALF), min(W - 1, cx + HALF)
        sc0 = x0 - (cx - HALF)
        ncols = x1 - x0 + 1
        for k in range(y0 >> 7, (y1 >> 7) + 1):
            t = cy - 128 * k + 14
            pieces.append((t, k, x0, ncols, sc0))
    return pieces


def _host_windows_and_banks(landmarks):
    """Per-core window piece lists (with dense shift indices) + stencil banks.

    bank[core][p, ti, j] = 299 * (1 - sqrt((p - t + 14)^2 + (j - 14)^2)/15)
    for t = shifts[core][ti]; negative outside the radius = max-neutral
    against the zero-initialized fields.
    """
    core_pieces = []  # [core][bi][field] -> list[(ti, k, x0, ncols, sc0)]
    core_shifts = []
    for core in range(NCORES):
        shifts = set()
        raw = []
        for bi in range(BPC):
            lm_b = landmarks[core * BPC + bi]
            per_field = []
            for lo, hi in (EYE, MOUTH):
                ps = _windows_for(lm_b, lo, hi)
                shifts.update(t for t, *_ in ps)
                per_field.append(ps)
            raw.append(per_field)
        slist = sorted(shifts)
        tmap = {t: i for i, t in enumerate(slist)}
        core_pieces.append(
            [
                [[(tmap[t], k, x0, nc_, sc0) for t, k, x0, nc_, sc0 in ps] for ps in pf]
                for pf in raw
            ]
        )
        core_shifts.append(slist)

    tmax = max(len(s) for s in core_shifts)
    banks = np.zeros((NCORES, 128, tmax, WIN), dtype=np.float32)
    p = np.arange(128.0)[:, None]
    j = np.arange(float(WIN))[None, :]
    for core, slist in enumerate(core_shifts):
        for ti, t in enumerate(slist):
            dist = np.sqrt((p - t + 14.0) ** 2 + (j - 14.0) ** 2)
            banks[core, :, ti, :] = (WEIGHT - 1.0) * (1.0 - dist / RADIUS)
    return core_pieces, banks.astype(mybir.dt.np(BF16)), tmax


def _build(core_pieces, tmax):
    """Build the SPMD Bass program, specialized to the landmark values."""
    nc = bacc.Bacc(None)
    pred_p = nc.declare_dram_parameter("pred", [BPC, C, H, W], FP32, isOutput=False)
    targ_p = nc.declare_dram_parameter("targ", [BPC, C, H, W], FP32, isOutput=False)
    bank_p = nc.declare_dram_parameter("bank", [128, tmax, WIN], BF16, isOutput=False)
    out_p = nc.declare_dram_parameter("out", [128, NU], FP32, isOutput=True)

    with tile.TileContext(nc) as tc, ExitStack() as ctx:
        stat_pool = ctx.enter_context(tc.tile_pool(name="stat", bufs=1))
        const_pool = ctx.enter_context(tc.tile_pool(name="const", bufs=1))
        load_pool = ctx.enter_context(tc.tile_pool(name="load", bufs=2))
        field_pool = ctx.enter_context(tc.tile_pool(name="field", bufs=2))

        # ---- tiles; fields zeroed on ACT (its queue is otherwise idle until
        # the Abs stream ~10us in) so the Q7 queue is pure DMA emission and
        # the casting loads start immediately ----
        tiles = []
        for bi in range(BPC):
            p_t = load_pool.tile([128, C, 4, W], BF16, tag="p_t", name=f"p_t{bi}")
            t_t = load_pool.tile([128, C, 4, W], BF16, tag="t_t", name=f"t_t{bi}")
            e_f = field_pool.tile([128, 4, W], BF16, tag="e_f", name=f"e_f{bi}")
            m_f = field_pool.tile([128, 4, W], BF16, tag="m_f", name=f"m_f{bi}")
            nc.scalar.memzero(e_f[:])
            nc.scalar.memzero(m_f[:])
            tiles.append((p_t, t_t, e_f, m_f))

        # ---- host-precomputed stencil bank: first on the sync ring so the
        # windows unblock immediately ----
        bank_t = const_pool.tile([128, tmax, WIN], BF16)
        nc.sync.dma_start(bank_t[:], bank_p[:])

        # ---- casting loads (SWDGE): fp32 HBM -> bf16 SBUF per chunk; the Q7
        # emits one DMA per ~1.1us, faster than the ~3.6us/chunk drain, so
        # the stream stays saturated once the first two chunks are queued ----
        for bi, k, x0, xn in UNITS:
            p_t, t_t, _, _ = tiles[bi]
            rows = slice(128 * k, 128 * (k + 1))
            cols = slice(x0, x0 + xn)
            nc.gpsimd.dma_start(
                p_t[:, :, k, cols],
                pred_p[bi, :, rows, cols].rearrange("c p x -> p c x"),
            )
            nc.gpsimd.dma_start(
                t_t[:, :, k, cols],
                targ_p[bi, :, rows, cols].rearrange("c p x -> p c x"),
            )

        # partition id + dispatch-prefetch hint AFTER the load issues
        core_idx = nc.vector.partition_id()
        win_hint = nc.vector.switch_hint(core_idx, NCORES, label="win")

        # pre-switch DVE touches of every cross-engine input the switch body
        # reads (fields from Q7 memsets, bank from the sync DMA): the waits
        # land here, outside the branch, and are elided inside it -- a
        # cross-engine wait inside a Switch case deadlocks the scheduler
        touch = stat_pool.tile([128, 1], BF16)
        for bi in range(BPC):
            _, _, e_f, m_f = tiles[bi]
            nc.vector.tensor_tensor(
                touch[:], e_f[:, 0, 0:1], m_f[:, 0, 0:1], op=Alu.max
            )
        nc.vector.tensor_tensor(touch[:], touch[:], bank_t[:, 0, 0:1], op=Alu.max)

        # ---- per-core landmark windows, one Switch for both batches ----
        for case in tc.Switch(core_idx, NCORES, hint=win_hint):
            for bi in range(BPC):
                _, _, e_f, m_f = tiles[bi]
                for field, pieces in zip((e_f, m_f), core_pieces[case][bi]):
                    for ti, k, x0, ncols, sc0 in pieces:
                        nc.vector.tensor_tensor(
                            field[:, k, x0 : x0 + ncols],
                            field[:, k, x0 : x0 + ncols],
                            bank_t[:, ti, sc0 : sc0 + ncols],
                            op=Alu.max,
                        )

        # ---- V = 1 + min(e'+m', 299), in place into e_f (DVE: the add runs
        # 2x, the fused min/+1 tensor_scalar runs 4x) ----
        v_ts = []
        for bi in range(BPC):
            _, _, e_f, m_f = tiles[bi]
            nc.vector.tensor_tensor(e_f[:], e_f[:], m_f[:], op=Alu.add)
            v_ts.append(nc.vector.tensor_scalar(
                e_f[:], e_f[:], WEIGHT - 1.0, 1.0, op0=Alu.min, op1=Alu.add
            ))

        # ---- per-unit pipeline: d = p-t (fp32 in, bf16 out); d *= V (bf16
        # 2x); |d| on ACT with fp32 row-sums ----
        rs = stat_pool.tile([128, NU], FP32)
        prev_mult = None
        for u, (bi, k, x0, xn) in enumerate(UNITS):
            p_t, t_t, e_f, _ = tiles[bi]
            cols = slice(x0, x0 + xn)
            dp = p_t[:, :, k, cols]
            sub = nc.vector.tensor_tensor(
                dp, dp, t_t[:, :, k, cols], op=Alu.subtract
            )
            if u >= NU - 2 and prev_mult is not None:
                # tail: pin unit-major order so the last ready MULTIPLY isn't
                # queued behind a data-blocked SUBTRACT
                add_dep_helper(sub.ins, prev_mult.ins, reason="unit-major tail")
            if u == 1:
                # force the windows->V chain for batch 0 to clear the DVE
                # queue before the unit stream takes over; otherwise V lands
                # late and every MULTIPLY serializes at the tail
                add_dep_helper(sub.ins, v_ts[0].ins, reason="V0 before units")
            if u == 5:
                add_dep_helper(sub.ins, v_ts[1].ins, reason="V1 before batch1")
            v3 = (
                e_f[:, k, cols]
                .broadcast_to([128, xn, C])
                .rearrange("p x c -> p c x")
            )
            prev_mult = nc.vector.tensor_tensor(dp, dp, v3, op=Alu.mult)
            nc.scalar.activation(dp, dp, Act.Abs, accum_out=rs[:, u : u + 1])
            if u == NU - 2:
                # all but the last tail column: overlap the big out-DMA
                nc.sync.dma_start(out_p[:, 0 : NU - 1], rs[:, 0 : NU - 1])
        nc.sync.dma_start(out_p[:, NU - 1 : NU], rs[:, NU - 1 : NU])

    return nc


def run(inputs, trace=False):
    pred = np.ascontiguousarray(inputs["pred"], dtype=np.float32)
    targ = np.ascontiguousarray(inputs["target"], dtype=np.float32)
    lms = np.asarray(inputs["landmarks"])
    assert pred.shape == (B, C, H, W) and targ.shape == (B, C, H, W)

    core_pieces, banks_bf16, tmax = _host_windows_and_banks(lms)
    nc = _build(core_pieces, tmax)
    nc.finalize()
    in_maps = [
        {
            "pred": pred[i * BPC : (i + 1) * BPC],
            "targ": targ[i * BPC : (i + 1) * BPC],
            "bank": banks_bf16[i],
        }
        for i in range(NCORES)
    ]
    res = run_bass_kernel_spmd(nc, in_maps, list(range(NCORES)), trace=trace)
    total = 0.0
    for i in range(NCORES):
        total += res.results[i]["out"].astype(np.float64).sum()
    return np.float32(total / NTOT), res


def kernel(pred, target, landmarks):
    out, _ = run({"pred": pred, "target": target, "landmarks": landmarks})
    return out
